# revision 6
# baseline (speedup 1.0000x reference)
"""Multi-head causal attention (B=2, L=2048, D=1024, H=16) on 8 trn2 cores.

Sharding: core c -> batch b=c//4, head-group g=c%4 (4 heads, 256 channels).
Dataflow (per core, zero on-device transposes; host feeds X^T per batch):
  K^T, Q^T = W.T @ X^T          [256ch, 2048rows]   (lhsT=W natural, rhs=X^T)
  V       = X @ W               [2048rows, 256ch]   (lhsT=X^T, rhs=W)
  S^T     = K^T.T @ Q^T slices  [128k, q]           (per head, causal tiles)
  E       = exp(S^T), diag-masked; P V via lhsT=V_aug (ones col -> sums row)
  O^T    /= sums  -> AllGather over the 4 cores of the batch -> O_cat^T
  out     = O_cat^T.T @ Wo[:, gslice]               (channel-split out-proj)
All matmuls run as float32r (tf32-like, 1 cyc/row at N>=256), fp32 PSUM accum.
"""

import numpy as np

import concourse.bass as bass
import concourse.mybir as mybir
import concourse.tile as tile
from concourse import bacc
from concourse.bass_utils import run_bass_kernel_spmd

B, L, D, H, DH = 2, 2048, 1024, 16, 64
NCORES = 8
G = 4          # head-groups (cores per batch)
CH = D // G    # channels per core (256) = 4 heads
HPC = 4        # heads per core
R = mybir.dt.float32r
F = mybir.dt.float32
AF = mybir.ActivationFunctionType
NEG = -1.0e30

RCHUNK = 512   # row chunk for projections / q-chunks
NRC = L // RCHUNK          # 4
NKT = L // 128             # 16 k-tiles
ND = D // 128              # 8 d-tiles

_cache = {}


def build():
    if "nc" in _cache:
        return _cache["nc"]
    nc = bacc.Bacc(trn_type="TRN2", num_devices=NCORES)

    xqt = nc.dram_tensor("xqt", [D, L], R, kind="ExternalInput")
    xkt = nc.dram_tensor("xkt", [D, L], R, kind="ExternalInput")
    xvt = nc.dram_tensor("xvt", [D, L], R, kind="ExternalInput")
    wq = nc.dram_tensor("wq", [D, CH], R, kind="ExternalInput")
    wk = nc.dram_tensor("wk", [D, CH], R, kind="ExternalInput")
    wv = nc.dram_tensor("wv", [D, CH], R, kind="ExternalInput")
    wo = nc.dram_tensor("wo", [D, CH], R, kind="ExternalInput")
    bq = nc.dram_tensor("bq", [1, CH], R, kind="ExternalInput")
    bk = nc.dram_tensor("bk", [1, CH], R, kind="ExternalInput")
    bv = nc.dram_tensor("bv", [1, CH], R, kind="ExternalInput")
    bo = nc.dram_tensor("bo", [1, CH], R, kind="ExternalInput")
    tri = nc.dram_tensor("tri", [128, 128], R, kind="ExternalInput")
    onesd = nc.dram_tensor("ones", [128, 512], R, kind="ExternalInput")
    out = nc.dram_tensor("out", [L, CH], F, kind="ExternalOutput")

    with tile.TileContext(nc) as tc:
        with (
            tc.tile_pool(name="const", bufs=1) as cpool,
            tc.tile_pool(name="wts", bufs=1) as wpool,
            tc.tile_pool(name="acts", bufs=1) as apool,
            tc.tile_pool(name="xin", bufs=2) as xpool,
            tc.tile_pool(name="eo", bufs=4) as epool,
            tc.tile_pool(name="ot", bufs=2) as opool,
            tc.tile_pool(name="ao", bufs=2) as dpool,
            tc.tile_pool(name="outs", bufs=3) as outpool,
            tc.tile_pool(name="small", bufs=4) as spool,
            tc.tile_pool(name="ps_s", bufs=2, space="PSUM") as pss,
            tc.tile_pool(name="ps_o", bufs=2, space="PSUM") as pso,
            tc.tile_pool(name="ps_pr", bufs=2, space="PSUM") as pspr,
            tc.tile_pool(name="dram", bufs=1, space="DRAM") as drpool,
        ):
            # ---- constants ----
            ones128 = cpool.tile([128, RCHUNK], R)
            nc.sync.dma_start(ones128[:], onesd[:])
            ones = ones128[0:1, :]
            tri_sb = cpool.tile([128, 128], R)
            nc.sync.dma_start(tri_sb[:], tri[:])
            b_sb = {}
            for nm, t in (("bq", bq), ("bk", bk), ("bv", bv), ("bo", bo)):
                b_sb[nm] = cpool.tile([1, CH], R, tag=nm, name=nm)
                nc.sync.dma_start(b_sb[nm][:], t[:])

            # ---- weights ----
            w_sb = {}
            for nm, t in (("q", wq), ("k", wk), ("v", wv), ("o", wo)):
                w_sb[nm] = []
                for d in range(ND):
                    wt = wpool.tile([128, CH], R, tag=f"w{nm}{d}", name=f"w{nm}{d}")
                    nc.sync.dma_start(wt[:], t[128 * d : 128 * d + 128, :])
                    w_sb[nm].append(wt)

            # ---- persistent activations ----
            kT = [apool.tile([128, L], R, tag=f"kT{t}", name=f"kT{t}") for t in range(2)]
            qT = [apool.tile([128, L], R, tag=f"qT{t}", name=f"qT{t}") for t in range(2)]
            v_sb = [apool.tile([128, HPC * 65], R, tag=f"v{i}", name=f"v{i}") for i in range(NKT)]

            # ---- stage A: projections, per row-chunk ----
            for j in range(NRC):
                rs = RCHUNK * j
                # K^T and Q^T: out [ch, rows]
                for nm, dst, xsrc, scale in (
                    ("k", kT, xkt, 1.0),
                    ("q", qT, xqt, 0.125),
                ):
                    xs = []
                    for d in range(ND):
                        xt = xpool.tile([128, RCHUNK], R, tag=f"x{d}", name=f"x{d}")
                        nc.sync.dma_start(
                            xt[:], xsrc[128 * d : 128 * d + 128, rs : rs + RCHUNK]
                        )
                        xs.append(xt)
                    for ct in range(2):
                        ps = pspr.tile([128, RCHUNK], F, tag="ps_pr", name="ps_pr")
                        for d in range(ND):
                            nc.tensor.matmul(
                                ps[:],
                                w_sb[nm][d][:, 128 * ct : 128 * ct + 128],
                                xs[d][:],
                                start=(d == 0),
                                stop=False,
                            )
                        nc.tensor.matmul(
                            ps[:],
                            b_sb["b" + nm][:, 128 * ct : 128 * ct + 128],
                            ones,
                            start=False,
                            stop=True,
                        )
                        nc.scalar.activation(
                            dst[ct][:, rs : rs + RCHUNK], ps[:], AF.Copy, scale=scale
                        )
                # V: out [rows, ch], lhsT = X^T slice
                xs = []
                for d in range(ND):
                    xt = xpool.tile([128, RCHUNK], R, tag=f"x{d}", name=f"x{d}")
                    nc.sync.dma_start(
                        xt[:], xvt[128 * d : 128 * d + 128, rs : rs + RCHUNK]
                    )
                    xs.append(xt)
                for rt in range(RCHUNK // 128):
                    i = (RCHUNK // 128) * j + rt
                    psv = pspr.tile([128, CH], F, tag="ps_pr", name="ps_pr")
                    for d in range(ND):
                        nc.tensor.matmul(
                            psv[:],
                            xs[d][:, 128 * rt : 128 * rt + 128],
                            w_sb["v"][d][:],
                            start=(d == 0),
                            stop=False,
                        )
                    nc.tensor.matmul(
                        psv[:], ones128[0:1, 0:128], b_sb["bv"][:], start=False, stop=True
                    )
                    nc.vector.tensor_copy(
                        v_sb[i][:].rearrange("p (h c) -> p h c", h=HPC)[:, :, 0:64],
                        psv[:].rearrange("p (h c) -> p h c", h=HPC),
                    )
                    nc.vector.tensor_copy(
                        v_sb[i][:].rearrange("p (h c) -> p h c", h=HPC)[:, :, 64:65],
                        ones128[:, 0:HPC].rearrange("p (h c) -> p h c", h=HPC),
                    )

            # ---- stages B-D per q-chunk ----
            ag_in = [drpool.tile([CH, RCHUNK], R, tag=f"agi{j}", name=f"agi{j}") for j in range(NRC)]
            ag_out = [drpool.tile([D, RCHUNK], R, tag=f"ago{j}", name=f"ago{j}") for j in range(NRC)]
            rg = [[0, 1, 2, 3], [4, 5, 6, 7]]

            for Jq in range(NRC):
                qs = RCHUNK * Jq
                nkt = 4 * Jq + 4  # causal k-tiles for this q-chunk
                oT_t = [opool.tile([128, RCHUNK], R, tag=f"oT{t}", name=f"oT{t}") for t in range(2)]
                for hh in range(HPC):
                    ct, po = hh // 2, 64 * (hh % 2)
                    ps_o = pso.tile([65, RCHUNK], F, tag="ps_o", name="ps_o")
                    for u in range(nkt // 2):
                        ps_s = pss.tile([128, 1024], F, tag="ps_s", name="ps_s")
                        e = epool.tile([128, 1024], R, tag="e", name="e")
                        for t in range(2):
                            i = 2 * u + t
                            rel = i - 4 * Jq
                            c0 = 128 * rel if rel > 0 else 0
                            if c0 > 0:
                                nc.vector.memset(ps_s[:, 512 * t : 512 * t + c0], NEG)
                            nc.tensor.matmul(
                                ps_s[:, 512 * t + c0 : 512 * t + 512],
                                kT[ct][po : po + 64, 128 * i : 128 * i + 128],
                                qT[ct][po : po + 64, qs + c0 : qs + RCHUNK],
                                start=True,
                                stop=True,
                            )
                        nc.scalar.activation(e[:], ps_s[:], AF.Exp)
                        for t in range(2):
                            i = 2 * u + t
                            rel = i - 4 * Jq
                            if rel >= 0:
                                c0 = 512 * t + 128 * rel
                                nc.vector.tensor_mul(
                                    e[:, c0 : c0 + 128],
                                    e[:, c0 : c0 + 128],
                                    tri_sb[:],
                                )
                        for t in range(2):
                            i = 2 * u + t
                            nc.tensor.matmul(
                                ps_o[:],
                                v_sb[i][:, 65 * hh : 65 * hh + 65],
                                e[:, 512 * t : 512 * t + 512],
                                start=(i == 0),
                                stop=(i == nkt - 1),
                            )
                    rec = spool.tile([1, RCHUNK], R, tag="rec", name="rec")
                    with nc.allow_low_precision(reason="f32r bits == f32 bits"):
                        nc.vector.reciprocal(rec[:], ps_o[64:65, :])
                    ps_b = pspr.tile([64, RCHUNK], F, tag="ps_pr", name="ps_pr")
                    nc.tensor.matmul(ps_b[:], ones128[0:1, 0:64], rec[:], start=True, stop=True)
                    bc_sb = spool.tile([64, RCHUNK], R, tag="bc", name="bc")
                    nc.vector.tensor_copy(bc_sb[:], ps_b[:])
                    nc.vector.tensor_mul(
                        oT_t[ct][po : po + 64, :], ps_o[0:64, :], bc_sb[:]
                    )
                for t in range(2):
                    nc.sync.dma_start(
                        ag_in[Jq][128 * t : 128 * t + 128, :], oT_t[t][:]
                    )
                nc.gpsimd.collective_compute(
                    "AllGather",
                    mybir.AluOpType.bypass,
                    replica_groups=rg,
                    ins=[ag_in[Jq].opt()],
                    outs=[ag_out[Jq].opt()],
                )
                # out-proj for rows of this chunk, channel slice
                ao = []
                for c in range(ND):
                    at = dpool.tile([128, RCHUNK], R, tag=f"ao{c}", name=f"ao{c}")
                    nc.sync.dma_start(at[:], ag_out[Jq][128 * c : 128 * c + 128, :])
                    ao.append(at)
                for rt in range(RCHUNK // 128):
                    ps = pspr.tile([128, CH], F, tag="ps_pr", name="ps_pr")
                    for c in range(ND):
                        nc.tensor.matmul(
                            ps[:],
                            ao[c][:, 128 * rt : 128 * rt + 128],
                            w_sb["o"][c][:],
                            start=(c == 0),
                            stop=False,
                        )
                    nc.tensor.matmul(
                        ps[:], ones128[0:1, 0:128], b_sb["bo"][:], start=False, stop=True
                    )
                    ot = outpool.tile([128, CH], F, tag="outt", name="outt")
                    nc.vector.tensor_copy(ot[:], ps[:])
                    nc.sync.dma_start(
                        out[qs + 128 * rt : qs + 128 * rt + 128, :], ot[:]
                    )

    nc.compile()
    _cache["nc"] = nc
    return nc


def make_in_maps(query, key, value, Wq, bq, Wk, bk, Wv, bv, Wo, bo):
    tri = np.triu(np.ones((128, 128), dtype=np.float32))
    in_maps = []
    xT = {}
    for b in range(B):
        xT[b] = (
            np.ascontiguousarray(np.asarray(query[b], np.float32).T),
            np.ascontiguousarray(np.asarray(key[b], np.float32).T),
            np.ascontiguousarray(np.asarray(value[b], np.float32).T),
        )
    for c in range(NCORES):
        b, g = c // G, c % G
        sl = slice(CH * g, CH * g + CH)
        xq, xk, xv = xT[b]
        in_maps.append(
            {
                "xqt": xq,
                "xkt": xk,
                "xvt": xv,
                "wq": np.ascontiguousarray(np.asarray(Wq, np.float32)[:, sl]),
                "wk": np.ascontiguousarray(np.asarray(Wk, np.float32)[:, sl]),
                "wv": np.ascontiguousarray(np.asarray(Wv, np.float32)[:, sl]),
                "wo": np.ascontiguousarray(np.asarray(Wo, np.float32)[:, sl]),
                "bq": np.asarray(bq, np.float32)[sl].reshape(1, CH),
                "bk": np.asarray(bk, np.float32)[sl].reshape(1, CH),
                "bv": np.asarray(bv, np.float32)[sl].reshape(1, CH),
                "bo": np.asarray(bo, np.float32)[sl].reshape(1, CH),
                "tri": tri,
                "ones": np.ones((128, 512), dtype=np.float32),
            }
        )
    return in_maps


def kernel(**inputs) -> np.ndarray:
    nc = build()
    in_maps = make_in_maps(**inputs)
    res = run_bass_kernel_spmd(nc, in_maps, core_ids=list(range(NCORES)))
    out = np.empty((B, L, D), dtype=np.float32)
    for c in range(NCORES):
        b, g = c // G, c % G
        out[b, :, CH * g : CH * g + CH] = res.results[c]["out"]
    return out
